# revision 17
# baseline (speedup 1.0000x reference)
"""Causal self-attention kernel for Trainium2, 8 NeuronCores.

Sharding: DP4 x TP2. Core c = 2*b + g handles batch b (2048 tokens) and
head-group g (8 of 16 heads). Per core:
  - x arrives pre-transposed AND pre-cast to bf16 on the host (d_model on
    partitions), so startup is plain parallel DMA (no xbar transposes);
    the first q-tile's x slices and the Q columns of w_qkv are DMA'd first
    so the PE can start within a few microseconds,
  - QKV matmuls in bf16: Q,K dim-major ([head_dim, tokens]), V token-major
    65 wide (64 dims + a ones column -> softmax denominator for free),
  - attention per head pair: scores^T = K_h^T-tile @ Q_h in [k, q] layout
    with both heads' QK matmuls in different PE row groups (concurrent),
    one wide exp on ACT (1/sqrt(64) folded into its scale) into bf16 probs,
    causal handling by skipping fully-masked tiles, sub-tile column ranges
    on the diagonal band (scores, exp AND att@V all restricted to off:512),
    a 0/1 mask multiply on the 128-wide diagonal band,
  - the attention inner loop is software-pipelined (scores for c-tile n+1
    are emitted before att@V for tile n) and each head pair's epilogue is
    held back until the NEXT head pair's first scores have been issued, so
    the in-order PE queue never blocks on ACT or on the epilogue chain,
  - the epilogue never touches ACT: DVE copies the denominator row
    (lane-aligned at partition 64), DVE fast-reciprocal, gpsimd partition
    broadcast, DVE scale into bf16 dim-major yT,
  - QKV for later token tiles and projection partials for earlier tiles
    are zipped between attention units to keep the PE warm while ACT
    drains the exps (the last, largest attention tile gets two projection
    tiles' worth of fill so the PE never idles into a HAM re-throttle),
  - bf16 projection partials are summed across the core pair with chunked
    ReduceScatter (pipelined behind attention; the final q-tile is one
    bigger chunk so only a single collective sits in the tail),
  - host assembles the 4x2048x1024 output from the 8 interleaved shards.

Everything (shapes, sharding) is hardcoded for
x: [4, 2048, 1024], w_qkv: [1024, 3072], w_proj: [1024, 1024], f32.
"""

import ml_dtypes
import numpy as np

import concourse.bacc as bacc
import concourse.mybir as mybir
import concourse.tile as tile
from concourse.tile import add_dep_helper
from concourse.bass_utils import run_bass_kernel_spmd

F32 = mybir.dt.float32
BF16 = mybir.dt.bfloat16

S = 2048  # tokens per core (one batch element)
D = 1024  # d_model
HL = 8  # heads per core (local)
HD = 64  # head dim
GD = HL * HD  # 512, head-group dim
VW = HD + 1  # V row width: 64 dims + ones column (denominator)
NQT = S // 512  # 4 q-tiles of 512
NDM = D // 128  # 8 d_model chunks
NTOK = S // 128  # 16 token tiles of 128
RG = [[0, 1], [2, 3], [4, 5], [6, 7]]
# ReduceScatter chunks (start_row, n_rows): one 512-row chunk per q-tile —
# the ~12us fixed cost per collective dominates, so fewer+larger wins, and
# spacing triggers one attention tile apart keeps the CC queue drained so
# gpsimd doorbell writes never block the epilogue broadcasts behind them
CHUNKS = [(k * 512, 512) for k in range(4)]

_NC_CACHE = {}


def _qkv_units(nc, P, n):
    """QKV matmul chains for token tile n, as separately emittable units."""
    units = []

    def qk_chain(m):
        def emit():
            ps = P.b1_ps.tile([128, 512], F32, tag="b1", name="qkps")
            for k in range(NDM):
                nc.tensor.matmul(
                    ps,
                    P.w_sb[:, k, m * 128 : (m + 1) * 128],
                    P.xT[:, k, n * 512 : (n + 1) * 512],
                    start=(k == 0),
                    stop=(k == NDM - 1),
                )
            nc.vector.tensor_copy(
                out=P.qkT[:, m, n * 512 : (n + 1) * 512], in_=ps
            )

        return emit

    def v_chain(t4):
        def emit():
            t = n * 4 + t4
            ps = P.b1_ps.tile([128, 512], F32, tag="b1", name="vps")
            for k in range(NDM):
                nc.tensor.matmul(
                    ps,
                    P.xT[:, k, t * 128 : (t + 1) * 128],
                    P.w_sb[:, k, 2 * GD : 3 * GD],
                    start=(k == 0),
                    stop=(k == NDM - 1),
                )
            nc.vector.tensor_copy(
                out=P.v_sb[:, t, :, 0:HD],
                in_=ps.rearrange("p (h d) -> p h d", h=HL),
            )

        return emit

    for m in range(2 * GD // 128):
        units.append(qk_chain(m))
    for t4 in range(4):
        units.append(v_chain(t4))
    return units


def _attn_units(nc, P, j, pending):
    """Attention units for q-tile j, software-pipelined per head pair.
    Each head pair's epilogue is deferred into the next head pair's head
    (after its first two score units) via the `pending` 1-slot box."""
    units = []
    ncol = 4 * j + 4
    for hp in range(HL // 2):
        state = {}

        def alloc(state=state):
            state["yps"] = P.y_ps.tile(
                [128, 2, 512], F32, tag="yps", name="yps", bufs=1
            )

        def sc(c, hp=hp, state=state):
            def emit():
                d = c - 4 * j  # >= 0 on the diagonal band
                off = max(d, 0) * 128  # columns below off are fully masked
                sps2 = P.attn_ps.tile(
                    [128, 2, 512], F32, tag="sps2", name="sps2"
                )
                for hi in range(2):
                    h = 2 * hp + hi
                    po = (h % 2) * 64
                    nc.tensor.matmul(
                        sps2[:, hi, off:512],
                        P.qkT[po : po + 64, 4 + h // 2, c * 128 : (c + 1) * 128],
                        P.qkT[po : po + 64, h // 2, j * 512 + off : (j + 1) * 512],
                        start=True,
                        stop=True,
                    )
                probs2 = P.probs_p.tile(
                    [128, 2, 512], BF16, tag="probs", name="probs"
                )
                nc.scalar.activation(
                    out=probs2[:, :, off:512],
                    in_=sps2[:, :, off:512],
                    func=mybir.ActivationFunctionType.Exp,
                    scale=0.125,
                )
                if d >= 0:
                    for hi in range(2):
                        nc.vector.tensor_mul(
                            probs2[:, hi, off : off + 128],
                            probs2[:, hi, off : off + 128],
                            P.mask_sb,
                        )
                state[c] = (probs2, off)

            return emit

        def av(c, hp=hp, state=state):
            def emit():
                probs2, off = state.pop(c)
                yps = state["yps"]
                for hi in range(2):
                    h = 2 * hp + hi
                    nc.tensor.matmul(
                        yps[0:VW, hi, off:512],
                        P.v_sb[:, c, h, :],
                        probs2[:, hi, off:512],
                        start=(c == 0),
                        stop=(c == ncol - 1),
                    )

            return emit

        def epilogue(hp=hp, state=state, j=j):
            # ones-row out of PSUM to partition 0 (ACT can shift
            # partitions), DVE fast reciprocal, gpsimd partition broadcast,
            # DVE scale into bf16 dim-major yT
            yps = state["yps"]
            dsb = P.den_p.tile([1, 2, 512], F32, tag="den", name="den")
            nc.scalar.activation(
                out=dsb,
                in_=yps[HD : HD + 1, :, :],
                func=mybir.ActivationFunctionType.Copy,
            )
            nc.vector.reciprocal_approx_fast(out=dsb, in_=dsb)
            denb = P.den_p.tile([HD, 2, 512], F32, tag="denb", name="denb")
            nc.gpsimd.partition_broadcast(denb, dsb)
            for hi in range(2):
                h = 2 * hp + hi
                po = (h % 2) * 64
                P.anchor = nc.vector.tensor_mul(
                    P.yT[po : po + 64, h // 2, j * 512 : (j + 1) * 512],
                    yps[0:HD, hi, :],
                    denb[:, hi, :],
                )

        units.append(alloc)
        units.append(sc(0))
        units.append(sc(1))
        if pending[0] is not None:
            units.append(pending[0])
            pending[0] = None
        units.append(av(0))
        for c in range(2, ncol):
            units.append(sc(c))
            units.append(av(c - 1))
        units.append(av(ncol - 1))
        pending[0] = epilogue
    return units


def _proj_units(nc, P, j):
    """bf16 projection partials for q-tile j -> cc_in rows, 2 units/tile."""
    units = []
    for mt in range(4 * j, 4 * j + 4):
        state = {}

        def half(nh, mt=mt, state=state):
            def emit():
                if nh == 0:
                    state["osb"] = P.out_p.tile(
                        [128, D], F32, tag="osb", name="osb"
                    )
                osb = state["osb"]
                ps = P.b1_ps.tile([128, 512], F32, tag="b1", name="ops")
                for kk in range(GD // 128):
                    nc.tensor.matmul(
                        ps,
                        P.yT[:, kk, mt * 128 : (mt + 1) * 128],
                        P.wp_sb[:, kk, nh * 512 : (nh + 1) * 512],
                        start=(kk == 0),
                        stop=(kk == GD // 128 - 1),
                    )
                nc.vector.tensor_copy(
                    out=osb[:, nh * 512 : (nh + 1) * 512], in_=ps
                )
                if nh == 1:
                    nc.sync.dma_start(
                        out=P.cc_in[mt * 128 : (mt + 1) * 128, :], in_=osb
                    )

            return emit

        units.append(half(0))
        units.append(half(1))
    return units


def _rs_unit(nc, P, k):
    """ReduceScatter of chunk k in fp32 -> cc_red (collectives may not
    write IO tensors; a plain DRAM-to-DRAM DMA moves the shard out in the
    tail with no cast stage)."""

    def emit():
        lo, n = CHUNKS[k]
        nc.gpsimd.collective_compute(
            "ReduceScatter",
            mybir.AluOpType.add,
            replica_groups=RG,
            ins=[P.cc_in[lo : lo + n, :].opt()],
            outs=[P.cc_red[lo // 2 : lo // 2 + n // 2, :].opt()],
        )

    return emit


def _out_unit(nc, P, k):
    """DRAM->DRAM copy of my fp32 shard of chunk k into the output tensor.
    Anchored after P.anchor so the compile-time scheduler (optimistic about
    collective latency) cannot hoist the wait into the attention region."""

    def emit():
        lo, n = CHUNKS[k]
        rows = slice(lo // 2, lo // 2 + n // 2)
        cp = nc.sync.dma_start(out=P.out[rows, :], in_=P.cc_red[rows, :])
        if P.anchor is not None:
            add_dep_helper(
                cp.ins, P.anchor.ins, sync=True, reason="out copies stay in tail"
            )

    return emit


def _zip(a_units, fill_units):
    """Spread fill_units evenly between a_units (fills go to the PE's idle
    slots while ACT drains the exps)."""
    na, nf = len(a_units), len(fill_units)
    if nf == 0:
        return list(a_units)
    pos = [int(na * (f + 1) / (nf + 1)) for f in range(nf)]
    out = []
    fi = 0
    for i, u in enumerate(a_units):
        out.append(u)
        while fi < nf and pos[fi] <= i:
            out.append(fill_units[fi])
            fi += 1
    out.extend(fill_units[fi:])
    return out


class _Ctx:
    pass


def _build_nc():
    nc = bacc.Bacc(None, num_devices=8)
    P = _Ctx()

    xTd = nc.dram_tensor("xT", [D, S], BF16, kind="ExternalInput").ap()
    wqkv = nc.dram_tensor("wqkv", [D, 3 * GD], BF16, kind="ExternalInput").ap()
    wproj = nc.dram_tensor("wproj", [GD, D], BF16, kind="ExternalInput").ap()
    masks = nc.dram_tensor("masks", [128, 128], BF16, kind="ExternalInput").ap()
    P.out = nc.dram_tensor("out", [S // 2, D], F32, kind="ExternalOutput").ap()

    with tile.TileContext(nc) as tc:
        with (
            tc.tile_pool(name="const", bufs=1) as const,
            tc.tile_pool(name="w_p", bufs=1) as w_p,
            tc.tile_pool(name="big_p", bufs=1) as big_p,
            tc.tile_pool(name="probs_p", bufs=8) as probs_p,
            tc.tile_pool(name="den_p", bufs=2) as den_p,
            tc.tile_pool(name="out_p", bufs=2) as out_p,
            tc.tile_pool(name="b1_ps", bufs=2, space="PSUM") as b1_ps,
            tc.tile_pool(name="attn_ps", bufs=2, space="PSUM") as attn_ps,
            tc.tile_pool(name="y_ps", bufs=1, space="PSUM") as y_ps,
            tc.tile_pool(name="dram", bufs=1, space="DRAM") as dram,
        ):
            P.probs_p, P.den_p, P.out_p = probs_p, den_p, out_p
            P.b1_ps, P.attn_ps, P.y_ps = b1_ps, attn_ps, y_ps

            # Startup: plain DMAs only, whole k-chunks (contiguous DRAM, 3-4KB
            # partition lines -> full DMA bandwidth; column-sliced loads would
            # degrade to 1KB lines at ~6x lower throughput), k-interleaved so
            # QKV chain m=0 can start as soon as the first chunks land.
            P.xT = big_p.tile([128, NDM, S], BF16, name="xT")
            P.w_sb = w_p.tile([128, NDM, 3 * GD], BF16, name="w_sb")
            P.mask_sb = const.tile([128, 128], BF16, name="mask_sb")
            nc.sync.dma_start(out=P.mask_sb, in_=masks)
            for k in range(NDM):
                nc.sync.dma_start(
                    out=P.xT[:, k, :], in_=xTd[k * 128 : (k + 1) * 128, :]
                )
                nc.sync.dma_start(
                    out=P.w_sb[:, k, :], in_=wqkv[k * 128 : (k + 1) * 128, :]
                )
            # preload the exp table while DMAs run
            aw = const.tile([1, 2], F32, name="actwarm")
            nc.vector.memset(aw, 0.0)
            nc.scalar.activation(
                out=aw, in_=aw, func=mybir.ActivationFunctionType.Exp
            )
            P.wp_sb = w_p.tile([128, GD // 128, D], BF16, name="wp_sb")
            for kk in range(GD // 128):
                nc.sync.dma_start(
                    out=P.wp_sb[:, kk, :], in_=wproj[kk * 128 : (kk + 1) * 128, :]
                )

            P.qkT = big_p.tile([128, 2 * GD // 128, S], BF16, name="qkT")
            P.v_sb = big_p.tile([128, NTOK, HL, VW], BF16, name="v_sb")
            nc.vector.memset(P.v_sb[:, :, :, HD : HD + 1], 1.0)
            P.yT = big_p.tile([128, GD // 128, S], BF16, name="yT")

            P.cc_in = dram.tile([S, D], F32, name="cc_in")
            P.cc_red = dram.tile([S // 2, D], F32, name="cc_red")
            # tiny warm-up collective: prepays the ~11us first-op cost of
            # the CC stream long before the first real ReduceScatter
            ccw_i = dram.tile([2, 128], F32, name="ccw_i")
            ccw_o = dram.tile([1, 128], F32, name="ccw_o")
            wsb = const.tile([2, 128], F32, name="wsb")
            nc.vector.memset(wsb, 0.0)
            nc.sync.dma_start(out=ccw_i[0:2], in_=wsb)
            nc.gpsimd.collective_compute(
                "ReduceScatter",
                mybir.AluOpType.add,
                replica_groups=RG,
                ins=[ccw_i[0:2].opt()],
                outs=[ccw_o[0:1].opt()],
            )

            # QKV tile 0: only what attention (j0, hp0) needs up front; the
            # remaining chains become fill for the j0 attention region.
            q0 = _qkv_units(nc, P, 0)
            q0_lead = [q0[0], q0[4], q0[8], q0[9], q0[10], q0[11]]
            q0_rest = [q0[1], q0[5], q0[2], q0[6], q0[3], q0[7]]
            for u in q0_lead:
                u()

            pending = [None]
            P.anchor = None
            fills_by_tile = {
                0: lambda: q0_rest + _qkv_units(nc, P, 1),
                1: lambda: _qkv_units(nc, P, 2),
                2: lambda: _qkv_units(nc, P, 3),
                3: lambda: (
                    _proj_units(nc, P, 0)
                    + [_rs_unit(nc, P, 0)]
                    + _proj_units(nc, P, 1)
                    + [_rs_unit(nc, P, 1)]
                    + _proj_units(nc, P, 2)
                    + [_rs_unit(nc, P, 2)]
                ),
            }
            for j in range(NQT):
                a_units = _attn_units(nc, P, j, pending)
                for u in _zip(a_units, fills_by_tile[j]()):
                    u()

            # tail: final epilogue (sets the cast anchor), last q-tile's
            # projection, final ReduceScatter, then all casts (chunks 0-2
            # are long since reduced; their upcast+out-DMA hides under the
            # final collective)
            pending[0]()
            pending[0] = None
            for u in _proj_units(nc, P, 3):
                u()
            _rs_unit(nc, P, 3)()
            for k in range(4):
                _out_unit(nc, P, k)()

    nc.compile()
    return nc


def _host_consts():
    ki = np.arange(128)[:, None]
    qj = np.arange(128)[None, :]
    masks = (qj >= ki).astype(ml_dtypes.bfloat16)  # [128, 128] diagonal band
    return masks


def _in_maps(x, w_qkv, w_proj):
    masks = _host_consts()
    xT = {}
    wq16 = {}
    wp16 = {}
    maps = []
    for c in range(8):
        b, g = c // 2, c % 2
        if b not in xT:
            xT[b] = np.ascontiguousarray(x[b].T).astype(ml_dtypes.bfloat16)
        if g not in wq16:
            wq = w_qkv[:, g * GD : (g + 1) * GD]
            wk = w_qkv[:, D + g * GD : D + (g + 1) * GD]
            wv = w_qkv[:, 2 * D + g * GD : 2 * D + (g + 1) * GD]
            wq16[g] = np.ascontiguousarray(
                np.concatenate([wq, wk, wv], axis=1)
            ).astype(ml_dtypes.bfloat16)
            wp16[g] = np.ascontiguousarray(
                w_proj[g * GD : (g + 1) * GD, :]
            ).astype(ml_dtypes.bfloat16)
        maps.append(
            {"xT": xT[b], "wqkv": wq16[g], "wproj": wp16[g], "masks": masks}
        )
    return maps


def kernel(x, w_qkv, w_proj):
    x = np.ascontiguousarray(x, dtype=np.float32)
    w_qkv = np.ascontiguousarray(w_qkv, dtype=np.float32)
    w_proj = np.ascontiguousarray(w_proj, dtype=np.float32)
    if "nc" not in _NC_CACHE:
        _NC_CACHE["nc"] = _build_nc()
    nc = _NC_CACHE["nc"]
    r = run_bass_kernel_spmd(nc, _in_maps(x, w_qkv, w_proj), list(range(8)))
    out = np.empty((4, S, D), np.float32)
    for b in range(4):
        for rk in range(2):
            o = r.results[2 * b + rk]["out"]
            for k, (lo, n) in enumerate(CHUNKS):
                h = n // 2
                out[b, lo + rk * h : lo + (rk + 1) * h] = o[
                    lo // 2 : lo // 2 + h
                ]
    return out
